# revision 1
# baseline (speedup 1.0000x reference)
"""Depthwise 9x9 same-padding conv (single shared kernel) on Trainium2.

Strategy (per NeuronCore, pure data-parallel over batch across 8 cores):
  - Treat each (b, c) image [256, 256] independently; 256 images per core.
  - Contract over image ROWS on the PE: for each horizontal tap v (9), one
    matmul with a banded Toeplitz weight matrix (built host-side from K)
    accumulating into PSUM:  out[i, j] += sum_u K[u, v] * X[i+u-4, j+v-4].
    The horizontal shift j+v-4 is an AP offset into a width-padded SBUF
    tile; the vertical band lives in the stationary lhsT.
  - A 256-row image splits into two 128-row SBUF tiles. Rows 0..123 come
    entirely from tile0 (top-clipped band), rows 132..255 from tile1
    (bottom-clipped band). The 8 cross-tile rows 124..131 of 8 images are
    batched into one [128, W] strip tile with block-diagonal band weights.
  - J images are packed per DMA/SBUF tile (host pre-transposed layout) so
    every main DMA is one large contiguous 2D transfer. Input DMAs ride the
    SP HWDGE ring, output DMAs the ACT HWDGE ring, edge DMAs SWDGE.
"""

import numpy as np
import ml_dtypes

import concourse.bass as bass
from concourse import bacc
import concourse.mybir as mybir
import concourse.tile as tile
from concourse.bass_utils import run_bass_kernel_spmd

N_CORES = 8
B, C, H, W = 32, 64, 256, 256
KS, PAD = 9, 4
BC = B // N_CORES          # batches per core
NIMG = BC * C              # images per core
WP = W + 2 * PAD           # padded width 264
MT = 124                   # main out-rows per half-tile (0..123 / 132..255)
EG = 8                     # images per edge-strip group
NGRP = NIMG // EG
J = 4                      # images packed per main DMA / SBUF tile
NBLK = NIMG // J

# float32r inputs/weights: full fp32 storage, PE streams it at bf16 rate for
# N>=256 (relaxed-precision matmul), fp32 PSUM accumulation + fp32 output.
IN_DT = mybir.dt.float32r
IN_NP = np.float32

LAST_RESULT = None         # test.py inspects this


def _build_weights(Kf):
    """Banded Toeplitz lhsT matrices from the 9x9 kernel Kf (float32).

    Wtop[v, i', i] = Kf[i'-i+4, v]   out rows 0..123   from X rows 0..127
    Wbot[v, i', m] = Kf[i'-m,   v]   out rows 132..255 from X rows 128..255
    Wedge[v, 16g+m+u, 8g+m] = Kf[u, v]  out rows 124..131 from X rows 120..135,
                                         8 images block-diagonal
    """
    Wtop = np.zeros((KS, 128, MT), np.float32)
    Wbot = np.zeros((KS, 128, MT), np.float32)
    Wedge = np.zeros((KS, 128, 8 * EG), np.float32)
    for v in range(KS):
        for i in range(MT):
            for u in range(KS):
                ip = i + u - PAD
                if 0 <= ip < 128:
                    Wtop[v, ip, i] = Kf[u, v]
                ipb = i + u
                if 0 <= ipb < 128:
                    Wbot[v, ipb, i] = Kf[u, v]
        for g in range(EG):
            for m in range(8):
                for u in range(KS):
                    Wedge[v, 16 * g + m + u, 8 * g + m] = Kf[u, v]
    return Wtop, Wbot, Wedge


def _build_nc(n_img=NIMG, xbufs=4, obufs=4, psbufs=5):
    n_blk = n_img // J
    n_grp = n_img // EG
    nc = bacc.Bacc("TRN2", target_bir_lowering=False)
    Xm = nc.dram_tensor("Xm", [n_blk, 2, 128, J * WP], IN_DT, kind="ExternalInput")
    Xe = nc.dram_tensor("Xe", [n_grp, 128, WP], IN_DT, kind="ExternalInput")
    Wt = nc.dram_tensor("Wt", [KS, 128, MT], IN_DT, kind="ExternalInput")
    Wb = nc.dram_tensor("Wb", [KS, 128, MT], IN_DT, kind="ExternalInput")
    We = nc.dram_tensor("We", [KS, 128, 8 * EG], IN_DT, kind="ExternalInput")
    Om = nc.dram_tensor(
        "Om", [n_blk, 2, MT, J * W], mybir.dt.float32, kind="ExternalOutput"
    )
    Oe = nc.dram_tensor(
        "Oe", [n_grp, 8 * EG, W], mybir.dt.float32, kind="ExternalOutput"
    )

    with tile.TileContext(nc) as tc:
        with (
            tc.tile_pool(name="wpool", bufs=1) as wpool,
            tc.tile_pool(name="xpool", bufs=xbufs) as xpool,
            tc.tile_pool(name="epool", bufs=2) as epool,
            tc.tile_pool(name="opool", bufs=obufs) as opool,
            tc.tile_pool(name="oepool", bufs=2) as oepool,
            tc.tile_pool(name="psum", bufs=psbufs, space="PSUM") as pspool,
            tc.tile_pool(name="psum_e", bufs=2, space="PSUM") as pepool,
        ):
            wt = wpool.tile([128, KS, MT], IN_DT)
            wb = wpool.tile([128, KS, MT], IN_DT)
            we = wpool.tile([128, KS, 8 * EG], IN_DT)
            nc.gpsimd.dma_start(out=wt[:], in_=Wt[:].rearrange("v p m -> p v m"))
            nc.gpsimd.dma_start(out=wb[:], in_=Wb[:].rearrange("v p m -> p v m"))
            nc.gpsimd.dma_start(out=we[:], in_=We[:].rearrange("v p m -> p v m"))

            for blk in range(n_blk):
                for half in range(2):
                    xt = xpool.tile([128, J * WP], IN_DT)
                    nc.sync.dma_start(out=xt[:], in_=Xm[blk, half])
                    ot = opool.tile([MT, J * W], mybir.dt.float32)
                    wsel = wt if half == 0 else wb
                    for j in range(J):
                        ps = pspool.tile([MT, W], mybir.dt.float32)
                        for v in range(KS):
                            nc.tensor.matmul(
                                ps[:],
                                wsel[:, v, :],
                                xt[:, j * WP + v : j * WP + v + W],
                                start=(v == 0),
                                stop=(v == KS - 1),
                            )
                        nc.vector.tensor_copy(ot[:, j * W : (j + 1) * W], ps[:])
                    nc.scalar.dma_start(out=Om[blk, half], in_=ot[:])

                if blk % (EG // J) == 0:
                    g = blk // (EG // J)
                    et = epool.tile([128, WP], IN_DT)
                    nc.gpsimd.dma_start(out=et[:], in_=Xe[g])
                    pse = pepool.tile([8 * EG, W], mybir.dt.float32)
                    for v in range(KS):
                        nc.tensor.matmul(
                            pse[:],
                            we[:, v, :],
                            et[:, v : v + W],
                            start=(v == 0),
                            stop=(v == KS - 1),
                        )
                    oe = oepool.tile([8 * EG, W], mybir.dt.float32)
                    nc.vector.tensor_copy(oe[:], pse[:])
                    nc.gpsimd.dma_start(out=Oe[g], in_=oe[:])
    nc.compile()
    return nc


def _prep_inputs(X):
    """Host prep: pad width, cast bf16, pack J images per tile row-block."""
    Xp = np.zeros((B * C, H, WP), IN_NP)
    Xp[:, :, PAD : PAD + W] = X.reshape(B * C, H, W)
    # main: [cores, blk, J, 2half, 128, WP] -> [cores, blk, 2, 128, J, WP]
    Xm = (
        Xp.reshape(N_CORES, NBLK, J, 2, 128, WP)
        .transpose(0, 1, 3, 4, 2, 5)
        .reshape(N_CORES, NBLK, 2, 128, J * WP)
    )
    Xm = np.ascontiguousarray(Xm)
    # edge strips: rows 120..135 of each image, 8 images stacked per group
    Xe = np.ascontiguousarray(
        Xp[:, 120:136, :].reshape(N_CORES, NGRP, 128, WP)
    )
    return Xm, Xe


def _assemble_output(res):
    """Reassemble [B, C, H, W] fp32 from per-core Om/Oe."""
    out = np.empty((N_CORES, NIMG, H, W), np.float32)
    for k in range(N_CORES):
        om = res.results[k]["Om"].reshape(NBLK, 2, MT, J, W)
        oe = res.results[k]["Oe"].reshape(NGRP * EG, 8, W)
        o = out[k].reshape(NBLK, J, H, W)
        o[:, :, 0:MT, :] = om[:, 0].transpose(0, 2, 1, 3)
        o[:, :, 132 : 132 + MT, :] = om[:, 1].transpose(0, 2, 1, 3)
        out[k][:, 124:132, :] = oe
    return out.reshape(B, C, H, W)


def kernel(X, K):
    global LAST_RESULT
    X = np.asarray(X)
    K = np.asarray(K)
    assert X.shape == (B, C, H, W) and K.shape == (1, 1, KS, KS)

    Xm, Xe = _prep_inputs(X)
    Wtop, Wbot, Wedge = _build_weights(K[0, 0].astype(np.float32))
    Wtop = Wtop.astype(IN_NP)
    Wbot = Wbot.astype(IN_NP)
    Wedge = Wedge.astype(IN_NP)

    nc = _build_nc()
    in_maps = [
        {"Xm": Xm[k], "Xe": Xe[k], "Wt": Wtop, "Wb": Wbot, "We": Wedge}
        for k in range(N_CORES)
    ]
    res = run_bass_kernel_spmd(nc, in_maps, core_ids=list(range(N_CORES)))
    LAST_RESULT = res
    return _assemble_output(res)



# revision 2
# speedup vs baseline: 1.3072x; 1.3072x over previous
"""Depthwise 9x9 same-padding conv (single shared kernel) on Trainium2.

Strategy (per NeuronCore, pure data-parallel over batch across 8 cores):
  - Contract over image ROWS on the PE with banded Toeplitz weights, as in
    the fp32r version, but run the PE in fp8 DoubleRow mode: each matmul
    carries TWO independent (weights, ifmap) k-tiles that accumulate into
    the same PSUM tile at 0.5 cycles per output column - 4x the fp32r MAC
    rate.
  - Accuracy: X and K are split hi/lo (X = hi + lo/16, K = Khi + Klo/16,
    both planes e4m3).  The three product terms hi*Khi, lo*Khi, hi*Klo
    (27 tap-slots per output tile) are packed into 14 DoubleRow matmuls:
      A_v  (v=0..8): k0 = lo  @tap v * band(Khi_v)/16, k1 = hi @tap v * band(Khi_v)
      B_k  (k=0..3): k0 = hi  @tap 2k * band(Klo_2k)/16, k1 = hi1 @tap 2k
                     (= hi @tap 2k+1) * band(Klo_2k+1)/16
      C:             k0 = lo  @tap 8 * band(Klo_8)/256, k1 = hi @tap 8 *
                     band(Klo_8)/16
    SBUF data planes are stored [lo, hi, hi1] (hi1 = hi shifted 1 col) so
    every pair is an adjacent-plane natural AP slice.
  - Main tiles are M=128 out rows over input row windows [0..127]/[128..255]
    (DoubleRow needs M % 16 == 0).  The 8 rows at the window boundary come
    out partial (band clipped); matmul cost doesn't depend on M, so they are
    computed anyway and discarded on the host.  Full values for out rows
    124..131 come from the batched edge-strip path (8 images x 16 input
    rows per 128-partition strip), which uses the same 14-matmul packing.
  - J images are packed per DMA/SBUF tile. Input DMAs ride the SP HWDGE
    ring, output DMAs the ACT HWDGE ring, edge DMAs SWDGE.
"""

import numpy as np
import ml_dtypes

import concourse.bass as bass
from concourse import bacc
import concourse.mybir as mybir
import concourse.tile as tile
from concourse.bass_utils import run_bass_kernel_spmd

N_CORES = 8
B, C, H, W = 32, 64, 256, 256
KS, PAD = 9, 4
BC = B // N_CORES          # batches per core
NIMG = BC * C              # images per core
WP = W + 2 * PAD           # padded width 264
EG = 8                     # images per edge-strip group
NGRP = NIMG // EG
J = 4                      # images packed per main DMA / SBUF tile
NBLK = NIMG // J

F8 = ml_dtypes.float8_e4m3
IN_DT = mybir.dt.float8e4

LAST_RESULT = None         # test.py inspects this


def _band(col, M, qmax=None):
    """Banded Toeplitz [128, M] from 9-tap column col: W[p, m] = col[p - m + 4].

    Main tiles: p, m index the same 128-row window (in-row = m + u - 4,
    u = p - m + 4 in 0..8); band is clipped at the window edge, which is
    exact at the image pad edges and leaves partial rows (discarded) at the
    window-internal boundary.
    """
    Wm = np.zeros((128, M), np.float32)
    for m in range(M):
        for u in range(KS):
            p = m + u - 4
            if 0 <= p < 128:
                Wm[p, m] = col[u]
    return Wm


def _eband(col):
    """Edge band [128, 64]: strip partition 16g+q = in-row 120+q of image g,
    out 8g+m = out-row 124+m of image g; q = m + u."""
    Wm = np.zeros((128, 8 * EG), np.float32)
    for g in range(EG):
        for m in range(8):
            for u in range(KS):
                Wm[16 * g + m + u, 8 * g + m] = col[u]
    return Wm


def _build_weights(Kf):
    """14 DoubleRow pair-weight tiles for main (M=128) and edge (M=64)."""
    Khi = Kf.astype(F8).astype(np.float32)
    Klo = (16.0 * (Kf - Khi)).astype(F8).astype(np.float32)

    def pairs(bandfn, M):
        out = np.zeros((14, 128, 2, M), np.float32)
        for v in range(KS):                      # A_v
            out[v, :, 0, :] = bandfn(Khi[:, v] / 16.0)
            out[v, :, 1, :] = bandfn(Khi[:, v])
        for k in range(4):                       # B_k
            out[9 + k, :, 0, :] = bandfn(Klo[:, 2 * k] / 16.0)
            out[9 + k, :, 1, :] = bandfn(Klo[:, 2 * k + 1] / 16.0)
        out[13, :, 0, :] = bandfn(Klo[:, 8] / 256.0)   # C
        out[13, :, 1, :] = bandfn(Klo[:, 8] / 16.0)
        return out.astype(F8)

    Wmain = pairs(lambda c: _band(c, 128), 128)
    Wedge = pairs(_eband, 8 * EG)
    return Wmain, Wedge


# (pair_kind, ifmap plane base, tap offset) per DoubleRow matmul:
#   plane base 0 -> planes (lo, hi), base 1 -> planes (hi, hi1)
PAIRS = [(v, 0, v) for v in range(KS)] + [(9 + k, 1, 2 * k) for k in range(4)] \
    + [(13, 0, 8)]


def _build_nc(n_img=NIMG, xbufs=4, obufs=4, psbufs=5):
    n_blk = n_img // J
    n_grp = n_img // EG
    nc = bacc.Bacc("TRN2", target_bir_lowering=False)
    Xm = nc.dram_tensor("Xm", [n_blk, 2, 128, 3, J * WP], IN_DT, kind="ExternalInput")
    Xe = nc.dram_tensor("Xe", [n_grp, 128, 3, WP], IN_DT, kind="ExternalInput")
    Wm = nc.dram_tensor("Wm", [14, 128, 2 * 128], IN_DT, kind="ExternalInput")
    We = nc.dram_tensor("We", [14, 128, 2 * 8 * EG], IN_DT, kind="ExternalInput")
    Om = nc.dram_tensor(
        "Om", [n_blk, 2, 128, J * W], mybir.dt.float32, kind="ExternalOutput"
    )
    Oe = nc.dram_tensor(
        "Oe", [n_grp, 8 * EG, W], mybir.dt.float32, kind="ExternalOutput"
    )

    with tile.TileContext(nc) as tc:
        with (
            tc.tile_pool(name="wpool", bufs=1) as wpool,
            tc.tile_pool(name="xpool", bufs=xbufs) as xpool,
            tc.tile_pool(name="epool", bufs=2) as epool,
            tc.tile_pool(name="opool", bufs=obufs) as opool,
            tc.tile_pool(name="oepool", bufs=2) as oepool,
            tc.tile_pool(name="psum", bufs=psbufs, space="PSUM") as pspool,
            tc.tile_pool(name="psum_e", bufs=2, space="PSUM") as pepool,
        ):
            wm = wpool.tile([128, 14, 2, 128], IN_DT)
            we = wpool.tile([128, 14, 2, 8 * EG], IN_DT)
            nc.gpsimd.dma_start(out=wm[:], in_=Wm[:].rearrange("q p m -> p q m"))
            nc.gpsimd.dma_start(out=we[:], in_=We[:].rearrange("q p m -> p q m"))

            for blk in range(n_blk):
                for half in range(2):
                    xt = xpool.tile([128, 3, J * WP], IN_DT)
                    nc.sync.dma_start(out=xt[:], in_=Xm[blk, half])
                    ot = opool.tile([128, J * W], mybir.dt.float32)
                    for j in range(J):
                        ps = pspool.tile([128, W], mybir.dt.float32)
                        for i, (q, pb, v) in enumerate(PAIRS):
                            nc.tensor.matmul(
                                ps[:],
                                wm[:, q],
                                xt[:, pb:pb + 2, j * WP + v : j * WP + v + W],
                                start=(i == 0),
                                stop=(i == len(PAIRS) - 1),
                                perf_mode=mybir.MatmulPerfMode.DoubleRow,
                            )
                        nc.vector.tensor_copy(ot[:, j * W : (j + 1) * W], ps[:])
                    nc.scalar.dma_start(out=Om[blk, half], in_=ot[:])

                if blk % (EG // J) == 0:
                    g = blk // (EG // J)
                    et = epool.tile([128, 3, WP], IN_DT)
                    nc.gpsimd.dma_start(out=et[:], in_=Xe[g])
                    pse = pepool.tile([8 * EG, W], mybir.dt.float32)
                    for i, (q, pb, v) in enumerate(PAIRS):
                        nc.tensor.matmul(
                            pse[:],
                            we[:, q],
                            et[:, pb:pb + 2, v : v + W],
                            start=(i == 0),
                            stop=(i == len(PAIRS) - 1),
                            perf_mode=mybir.MatmulPerfMode.DoubleRow,
                        )
                    oe = oepool.tile([8 * EG, W], mybir.dt.float32)
                    nc.vector.tensor_copy(oe[:], pse[:])
                    nc.gpsimd.dma_start(out=Oe[g], in_=oe[:])
    nc.compile()
    return nc


def _prep_inputs(X):
    """Host prep: pad width, fp8 hi/lo split, build [lo, hi, hi1] planes,
    pack J images per row-block."""
    Xf = X.reshape(B * C, H, W)
    hi8 = Xf.astype(F8)
    lo8 = (16.0 * (Xf - hi8.astype(np.float32))).astype(F8)

    planes = np.zeros((B * C, H, 3, WP), F8)
    planes[:, :, 0, PAD : PAD + W] = lo8
    planes[:, :, 1, PAD : PAD + W] = hi8
    planes[:, :, 2, PAD - 1 : PAD + W - 1] = hi8     # hi shifted left by 1

    # main: [cores, blk, J, 2half, 128, 3, WP] -> [cores, blk, 2, 128, 3, J, WP]
    Xm = (
        planes.reshape(N_CORES, NBLK, J, 2, 128, 3, WP)
        .transpose(0, 1, 3, 4, 5, 2, 6)
        .reshape(N_CORES, NBLK, 2, 128, 3, J * WP)
    )
    Xm = np.ascontiguousarray(Xm)
    # edge strips: rows 120..135 of each image, 8 images stacked per group
    Xe = np.ascontiguousarray(
        planes[:, 120:136].reshape(N_CORES, NGRP, 128, 3, WP)
    )
    return Xm, Xe


def _assemble_output(res):
    """Reassemble [B, C, H, W] fp32 from per-core Om/Oe, discarding the
    partial window-boundary rows of the main tiles."""
    out = np.empty((N_CORES, NIMG, H, W), np.float32)
    for k in range(N_CORES):
        om = res.results[k]["Om"].reshape(NBLK, 2, 128, J, W)
        oe = res.results[k]["Oe"].reshape(NGRP * EG, 8, W)
        o = out[k].reshape(NBLK, J, H, W)
        o[:, :, 0:124, :] = om[:, 0, 0:124].transpose(0, 2, 1, 3)
        o[:, :, 132:256, :] = om[:, 1, 4:128].transpose(0, 2, 1, 3)
        out[k][:, 124:132, :] = oe
    return out.reshape(B, C, H, W)


def kernel(X, K):
    global LAST_RESULT
    X = np.asarray(X, dtype=np.float32)
    K = np.asarray(K, dtype=np.float32)
    assert X.shape == (B, C, H, W) and K.shape == (1, 1, KS, KS)

    Xm, Xe = _prep_inputs(X)
    Wmain, Wedge = _build_weights(K[0, 0])

    nc = _build_nc()
    in_maps = [
        {
            "Xm": Xm[k],
            "Xe": Xe[k],
            "Wm": Wmain.reshape(14, 128, 2 * 128),
            "We": Wedge.reshape(14, 128, 2 * 8 * EG),
        }
        for k in range(N_CORES)
    ]
    res = run_bass_kernel_spmd(nc, in_maps, core_ids=list(range(N_CORES)))
    LAST_RESULT = res
    return _assemble_output(res)


# revision 5
# speedup vs baseline: 1.4042x; 1.0742x over previous
"""Depthwise 9x9 same-padding conv (single shared kernel) on Trainium2.

Strategy (per NeuronCore, pure data-parallel over batch across 8 cores):
  - Contract over image ROWS on the PE with banded Toeplitz weights, as in
    the fp32r version, but run the PE in fp8 DoubleRow mode: each matmul
    carries TWO independent (weights, ifmap) k-tiles that accumulate into
    the same PSUM tile at 0.5 cycles per output column - 4x the fp32r MAC
    rate.
  - Accuracy: X and K are split hi/lo (X = hi + lo/16, K = Khi + Klo/16,
    both planes e4m3).  The three product terms hi*Khi, lo*Khi, hi*Klo
    (27 tap-slots per output tile) are packed into 14 DoubleRow matmuls:
      A_v  (v=0..8): k0 = lo  @tap v * band(Khi_v)/16, k1 = hi @tap v * band(Khi_v)
      B_k  (k=0..3): k0 = hi  @tap 2k * band(Klo_2k)/16, k1 = hi1 @tap 2k
                     (= hi @tap 2k+1) * band(Klo_2k+1)/16
      C:             k0 = lo  @tap 8 * band(Klo_8)/256, k1 = hi @tap 8 *
                     band(Klo_8)/16
    SBUF data planes are stored [lo, hi, hi1] (hi1 = hi shifted 1 col) so
    every pair is an adjacent-plane natural AP slice.
  - Main tiles are M=128 out rows over input row windows [0..127]/[128..255]
    (DoubleRow needs M % 16 == 0).  The 8 rows at the window boundary come
    out partial (band clipped); matmul cost doesn't depend on M, so they are
    computed anyway and discarded on the host.  Full values for out rows
    124..131 come from the batched edge-strip path (8 images x 16 input
    rows per 128-partition strip), which uses the same 14-matmul packing.
  - J images are packed per DMA/SBUF tile. Input DMAs ride the SP HWDGE
    ring, output DMAs the ACT HWDGE ring, edge DMAs SWDGE.
"""

import numpy as np
import ml_dtypes

import concourse.bass as bass
from concourse import bacc
import concourse.mybir as mybir
import concourse.tile as tile
from concourse.bass_utils import run_bass_kernel_spmd

N_CORES = 8
B, C, H, W = 32, 64, 256, 256
KS, PAD = 9, 4
BC = B // N_CORES          # batches per core
NIMG = BC * C              # images per core
WP = W + 2 * PAD           # padded width 264
EG = 8                     # images per edge-strip group
NGRP = NIMG // EG
J = 4                      # images packed per main DMA / SBUF tile
NBLK = NIMG // J

F8 = ml_dtypes.float8_e4m3
IN_DT = mybir.dt.float8e4

LAST_RESULT = None         # test.py inspects this


def _band(col, M, qmax=None):
    """Banded Toeplitz [128, M] from 9-tap column col: W[p, m] = col[p - m + 4].

    Main tiles: p, m index the same 128-row window (in-row = m + u - 4,
    u = p - m + 4 in 0..8); band is clipped at the window edge, which is
    exact at the image pad edges and leaves partial rows (discarded) at the
    window-internal boundary.
    """
    Wm = np.zeros((128, M), np.float32)
    for m in range(M):
        for u in range(KS):
            p = m + u - 4
            if 0 <= p < 128:
                Wm[p, m] = col[u]
    return Wm


def _eband(col):
    """Edge band [128, 64]: strip partition 16g+q = in-row 120+q of image g,
    out 8g+m = out-row 124+m of image g; q = m + u."""
    Wm = np.zeros((128, 8 * EG), np.float32)
    for g in range(EG):
        for m in range(8):
            for u in range(KS):
                Wm[16 * g + m + u, 8 * g + m] = col[u]
    return Wm


# Max tolerable residual energy for dropping one K-correction column: the
# added error is ~6.3*sqrt(ce) absolute vs a ~1.0 absolute budget (2e-2 of
# max|out| ~ 51); 0.003 keeps the addition under ~0.35.
DROP_CE_MAX = 0.003


def _g_plan(Kf):
    """Choose which K-residual (G-term) columns to compute and how to pair
    them.  Returns (g_pairs, g_single) where g_pairs is a list of leading
    taps v (pairing taps v and v+1 via the hi/hi1 planes) and g_single is
    an optional lone tap (paired with the lo*Klo bonus on the lo plane).
    Dropping the lowest-energy EVEN column (if cheap enough) makes the
    remaining 8 columns pair perfectly, saving one matmul per tile."""
    Khi = Kf.astype(F8).astype(np.float32)
    resid = Kf - Khi
    ce = (resid ** 2).sum(axis=0)
    evens = [0, 2, 4, 6, 8]
    c = min(evens, key=lambda v: ce[v])
    if ce[c] <= DROP_CE_MAX:
        cols = [v for v in range(KS) if v != c]
        g_pairs = [cols[i] for i in range(0, 8, 2)]   # (v, v+1) adjacent pairs
        return g_pairs, None
    return [0, 2, 4, 6], 8


def _build_weights(Kf, g_pairs, g_single):
    """DoubleRow pair-weight tiles for main (M=128) and edge (M=64)."""
    Khi = Kf.astype(F8).astype(np.float32)
    Klo = (16.0 * (Kf - Khi)).astype(F8).astype(np.float32)
    npairs = KS + len(g_pairs) + (1 if g_single is not None else 0)

    def pairs(bandfn, M):
        out = np.zeros((npairs, 128, 2, M), np.float32)
        for v in range(KS):                      # A_v
            out[v, :, 0, :] = bandfn(Khi[:, v] / 16.0)
            out[v, :, 1, :] = bandfn(Khi[:, v])
        for k, v in enumerate(g_pairs):          # G pair (v, v+1)
            out[KS + k, :, 0, :] = bandfn(Klo[:, v] / 16.0)
            out[KS + k, :, 1, :] = bandfn(Klo[:, v + 1] / 16.0)
        if g_single is not None:                 # lone G + lo*Klo bonus
            out[-1, :, 0, :] = bandfn(Klo[:, g_single] / 256.0)
            out[-1, :, 1, :] = bandfn(Klo[:, g_single] / 16.0)
        return out.astype(F8)

    Wmain = pairs(lambda c: _band(c, 128), 128)
    Wedge = pairs(_eband, 8 * EG)
    return Wmain, Wedge


def _pair_schedule(g_pairs, g_single):
    """(pair_kind, ifmap plane base, tap offset) per DoubleRow matmul:
    plane base 0 -> planes (lo, hi), base 1 -> planes (hi, hi1)."""
    sched = [(v, 0, v) for v in range(KS)]
    sched += [(KS + k, 1, v) for k, v in enumerate(g_pairs)]
    if g_single is not None:
        sched += [(KS + len(g_pairs), 0, g_single)]
    return sched


def _build_nc(pair_sched, n_img=NIMG, xbufs=4, obufs=4, psbufs=6):
    npairs = len(pair_sched)
    n_blk = n_img // J
    n_grp = n_img // EG
    nc = bacc.Bacc("TRN2", target_bir_lowering=False)
    Xm = nc.dram_tensor("Xm", [n_blk, 2, 128, 3, J * WP], IN_DT, kind="ExternalInput")
    Xe = nc.dram_tensor("Xe", [n_grp, 128, 3, WP], IN_DT, kind="ExternalInput")
    Wm = nc.dram_tensor("Wm", [npairs, 128, 2 * 128], IN_DT, kind="ExternalInput")
    We = nc.dram_tensor("We", [npairs, 128, 2 * 8 * EG], IN_DT, kind="ExternalInput")
    Om = nc.dram_tensor(
        "Om", [n_blk, 2, 128, J * W], mybir.dt.float32, kind="ExternalOutput"
    )
    Oe = nc.dram_tensor(
        "Oe", [n_grp, 8 * EG, W], mybir.dt.float32, kind="ExternalOutput"
    )

    with tile.TileContext(nc) as tc:
        with (
            tc.tile_pool(name="wpool", bufs=1) as wpool,
            tc.tile_pool(name="xpool", bufs=xbufs) as xpool,
            tc.tile_pool(name="epool", bufs=2) as epool,
            tc.tile_pool(name="opool", bufs=obufs) as opool,
            tc.tile_pool(name="oepool", bufs=2) as oepool,
            tc.tile_pool(name="psum", bufs=psbufs, space="PSUM") as pspool,
            tc.tile_pool(name="psum_e", bufs=2, space="PSUM") as pepool,
        ):
            wm = wpool.tile([128, npairs, 2, 128], IN_DT)
            we = wpool.tile([128, npairs, 2, 8 * EG], IN_DT)
            nc.gpsimd.dma_start(out=wm[:], in_=Wm[:].rearrange("q p m -> p q m"))
            nc.gpsimd.dma_start(out=we[:], in_=We[:].rearrange("q p m -> p q m"))

            for blk in range(n_blk):
                for half in range(2):
                    xt = xpool.tile([128, 3, J * WP], IN_DT)
                    nc.sync.dma_start(out=xt[:], in_=Xm[blk, half])
                    ot = opool.tile([128, J * W], mybir.dt.float32)
                    for j in range(J):
                        ps = pspool.tile([128, W], mybir.dt.float32)
                        for i, (q, pb, v) in enumerate(pair_sched):
                            nc.tensor.matmul(
                                ps[:],
                                wm[:, q],
                                xt[:, pb:pb + 2, j * WP + v : j * WP + v + W],
                                start=(i == 0),
                                stop=(i == npairs - 1),
                                perf_mode=mybir.MatmulPerfMode.DoubleRow,
                            )
                        nc.vector.tensor_copy(ot[:, j * W : (j + 1) * W], ps[:])
                    nc.scalar.dma_start(out=Om[blk, half], in_=ot[:])

                if blk % (EG // J) == 0:
                    g = blk // (EG // J)
                    et = epool.tile([128, 3, WP], IN_DT)
                    nc.gpsimd.dma_start(out=et[:], in_=Xe[g])
                    pse = pepool.tile([8 * EG, W], mybir.dt.float32)
                    for i, (q, pb, v) in enumerate(pair_sched):
                        nc.tensor.matmul(
                            pse[:],
                            we[:, q],
                            et[:, pb:pb + 2, v : v + W],
                            start=(i == 0),
                            stop=(i == npairs - 1),
                            perf_mode=mybir.MatmulPerfMode.DoubleRow,
                        )
                    oe = oepool.tile([8 * EG, W], mybir.dt.float32)
                    nc.vector.tensor_copy(oe[:], pse[:])
                    nc.gpsimd.dma_start(out=Oe[g], in_=oe[:])
    nc.compile()
    return nc


def _prep_inputs(X):
    """Host prep: pad width, fp8 hi/lo split, build [lo, hi, hi1] planes,
    pack J images per row-block."""
    Xf = X.reshape(B * C, H, W)
    hi8 = Xf.astype(F8)
    lo8 = (16.0 * (Xf - hi8.astype(np.float32))).astype(F8)

    planes = np.zeros((B * C, H, 3, WP), F8)
    planes[:, :, 0, PAD : PAD + W] = lo8
    planes[:, :, 1, PAD : PAD + W] = hi8
    planes[:, :, 2, PAD - 1 : PAD + W - 1] = hi8     # hi shifted left by 1

    # main: [cores, blk, J, 2half, 128, 3, WP] -> [cores, blk, 2, 128, 3, J, WP]
    Xm = (
        planes.reshape(N_CORES, NBLK, J, 2, 128, 3, WP)
        .transpose(0, 1, 3, 4, 5, 2, 6)
        .reshape(N_CORES, NBLK, 2, 128, 3, J * WP)
    )
    Xm = np.ascontiguousarray(Xm)
    # edge strips: rows 120..135 of each image, 8 images stacked per group
    Xe = np.ascontiguousarray(
        planes[:, 120:136].reshape(N_CORES, NGRP, 128, 3, WP)
    )
    return Xm, Xe


def _assemble_output(res):
    """Reassemble [B, C, H, W] fp32 from per-core Om/Oe, discarding the
    partial window-boundary rows of the main tiles."""
    out = np.empty((N_CORES, NIMG, H, W), np.float32)
    for k in range(N_CORES):
        om = res.results[k]["Om"].reshape(NBLK, 2, 128, J, W)
        oe = res.results[k]["Oe"].reshape(NGRP * EG, 8, W)
        o = out[k].reshape(NBLK, J, H, W)
        o[:, :, 0:124, :] = om[:, 0, 0:124].transpose(0, 2, 1, 3)
        o[:, :, 132:256, :] = om[:, 1, 4:128].transpose(0, 2, 1, 3)
        out[k][:, 124:132, :] = oe
    return out.reshape(B, C, H, W)


def kernel(X, K):
    global LAST_RESULT
    X = np.asarray(X, dtype=np.float32)
    K = np.asarray(K, dtype=np.float32)
    assert X.shape == (B, C, H, W) and K.shape == (1, 1, KS, KS)

    Xm, Xe = _prep_inputs(X)
    g_pairs, g_single = _g_plan(K[0, 0])
    Wmain, Wedge = _build_weights(K[0, 0], g_pairs, g_single)
    pair_sched = _pair_schedule(g_pairs, g_single)
    npairs = len(pair_sched)

    nc = _build_nc(pair_sched)
    in_maps = [
        {
            "Xm": Xm[k],
            "Xe": Xe[k],
            "Wm": Wmain.reshape(npairs, 128, 2 * 128),
            "We": Wedge.reshape(npairs, 128, 2 * 8 * EG),
        }
        for k in range(N_CORES)
    ]
    res = run_bass_kernel_spmd(nc, in_maps, core_ids=list(range(N_CORES)))
    LAST_RESULT = res
    return _assemble_output(res)
